# revision 43
# baseline (speedup 1.0000x reference)
"""Trainium2 Bass kernel for nn_AttentionBlock (GroupNorm -> QKV -> MHA -> proj -> residual).

Contract: kernel(**inputs) takes the FULL unsharded inputs (numpy), returns the
FULL output. Internally: data-parallel over batch B=8 across 8 NeuronCores, one
batch element per core, single Bass/Tile program run SPMD via
run_bass_kernel_spmd.

Math (per batch element b, all shapes hardcoded):
  x:[512,1024(=32*32)]  ->  GroupNorm(32 groups)  ->  xn
  qkv = qkv_w @ xn + qkv_b          (per head h: q,k,v in [64,1024])
  S'_h[s,t] = (k_h[:,s]) . (q_h[:,t] * 1/8)       (scale^2 folded into q wts)
  a_h[c,t]  = sum_s v_h[c,s] exp(S'_h[s,t]) / den_h[t];  den = col-sum exp
  out = x + proj_w @ a + proj_b
All weights and constants travel in ONE packed bf16 DRAM tensor "w" (f32
sections stored bitcast) so each device call binds only two input buffers.

Device layout choices:
  - weights pre-transposed/permuted on host so every matmul operand is in its
    natural [K(partitions), free] layout; matmuls run in bf16 (fp32 PSUM accum)
  - scores computed transposed (S'[s,t]) so softmax denominator comes free as an
    extra lhsT "ones" column in the PV matmul (row 64 of the PV psum)
  - 1/den via DVE reciprocal_approx_fast, broadcast across partitions on GPSIMD
  - build_bass(iters=K) chains K dependent copies of the body through DRAM
    scratch (out_i -> x_{i+1}) for device-time benchmarking; iters=1 is the
    production kernel.
"""

import numpy as np
import ml_dtypes

import concourse.bacc as bacc
import concourse.tile as tile
from concourse import mybir
from concourse.bass_utils import run_bass_kernel_spmd

F32 = mybir.dt.float32
BF16 = mybir.dt.bfloat16
ALU = mybir.AluOpType
ACTF = mybir.ActivationFunctionType

C = 512
T = 1024
NH = 8
CH = 64
GROUPS = 32
EPS = 1e-5
N_CORES = 8

# packed "w" tensor column offsets (bf16 columns)
OFF_WQK = 0                      # [128, 4, 1024] bf16
OFF_WV = OFF_WQK + 4 * 1024      # [128, 4, 512] bf16
OFF_WP = OFF_WV + 4 * 512        # [128, 4, 512] bf16
OFF_BV = OFF_WP + 4 * 512        # [1, 512] bf16 (row 0 only)
OFF_GMT = OFF_BV + 512           # [128, 128] f32 (256 bf16 cols)
OFF_GMAT = OFF_GMT + 256         # [128, 8] f32 (16)
OFF_GNW = OFF_GMAT + 16          # [128, 4] f32 (8)
OFF_GNB = OFF_GNW + 8            # [128, 4] f32 (8)
OFF_BQK = OFF_GNB + 8            # [128, 8] f32 (16)
OFF_BP = OFF_BQK + 16            # [128, 4] f32 (8)
WCOLS = OFF_BP + 8


def build_bass(iters=1):
    nc = bacc.Bacc(
        "TRN2", target_bir_lowering=False, debug=False, enable_asserts=False,
        enable_partition_id=False,
    )

    d_x = nc.dram_tensor("x", [C, T], F32, kind="ExternalInput").ap()
    d_w = nc.dram_tensor("w", [128, WCOLS], BF16, kind="ExternalInput").ap()
    d_out = nc.dram_tensor("out", [C, T], F32, kind="ExternalOutput").ap()

    bufs = 1 if iters == 1 else 2
    with tile.TileContext(nc) as tc:
        with (
            tc.tile_pool(name="weights", bufs=1) as wp_pool,
            tc.tile_pool(name="state", bufs=bufs) as pp,
            tc.tile_pool(name="work", bufs=8) as wk,
            tc.tile_pool(name="tail", bufs=2) as tl,
            tc.tile_pool(name="psum", bufs=2, space="PSUM") as ps,
            tc.tile_pool(name="psacc", bufs=2, space="PSUM") as pa,
            tc.tile_pool(name="chain", bufs=2, space="DRAM") as dp,
        ):
            pools = (wp_pool, pp, wk, tl, ps, pa)
            cur_in = d_x
            for it in range(iters):
                if it == iters - 1:
                    cur_out = d_out
                else:
                    scratch = dp.tile([C, T], F32, tag="chain", name=f"ch{it}")
                    cur_out = scratch[:]
                _body(tc, pools, cur_in, d_w, cur_out)
                cur_in = cur_out
    nc.compile()
    return nc


BENCH_UNROLL = 4


def build_bass_bench(max_iters=512):
    """Bench-only program: runs the full kernel body BENCH_UNROLL * K times in
    one NEFF (K read at runtime from the "iters" input), each execution
    chained through DRAM scratch (out_i -> x_{i+1}).  Wall-clock difference
    between two K values divided by the body-execution difference is pure
    device time per kernel execution -- dispatch/tunnel overhead cancels
    exactly."""
    nc = bacc.Bacc(
        "TRN2", target_bir_lowering=False, debug=False, enable_asserts=False,
        enable_partition_id=False,
    )
    d_x = nc.dram_tensor("x", [C, T], F32, kind="ExternalInput").ap()
    d_w = nc.dram_tensor("w", [128, WCOLS], BF16, kind="ExternalInput").ap()
    d_it = nc.dram_tensor("iters", [1, 1], mybir.dt.uint32, kind="ExternalInput").ap()
    d_out = nc.dram_tensor("out", [C, T], F32, kind="ExternalOutput").ap()

    with tile.TileContext(nc) as tc:
        with (
            tc.tile_pool(name="weights", bufs=1) as wp_pool,
            tc.tile_pool(name="state", bufs=2) as pp,
            tc.tile_pool(name="work", bufs=8) as wk,
            tc.tile_pool(name="tail", bufs=2) as tl,
            tc.tile_pool(name="psum", bufs=2, space="PSUM") as ps,
            tc.tile_pool(name="psacc", bufs=2, space="PSUM") as pa,
            tc.tile_pool(name="chain", bufs=2, space="DRAM") as dp,
        ):
            pools = (wp_pool, pp, wk, tl, ps, pa)
            scratch = dp.tile([C, T], F32, tag="chain", name="chain")
            nc.sync.dma_start(scratch[:], d_x[:])
            tmp = nc.alloc_registers("tmp_iters", mybir.ALL_ENGINES)
            nc.regs_load(tmp, d_it[0:1, 0:1])
            end = nc.snap(tmp, donate=True, min_val=0, max_val=max_iters)
            with tc.For_i(0, end):
                cur = scratch[:]
                for u in range(BENCH_UNROLL):
                    if u == BENCH_UNROLL - 1:
                        nxt = scratch[:]
                    else:
                        nxt = dp.tile([C, T], F32, tag="mid", name=f"mid{u}")[:]
                    _body(tc, pools, cur, d_w, nxt)
                    cur = nxt
            nc.sync.dma_start(d_out[:], scratch[:])
    nc.compile()
    return nc


def _body(tc, pools, d_x, d_w, d_out):
    nc = tc.nc
    wl, pp, wk, tl, ps, pa = pools
    V = NH * 65  # 520: per-head [v(64) | ones(1)] columns

    # ---- persistent SBUF tiles + input DMAs ----
    # x chunks first on the SP/HWDGE ring (the GN chain gates everything);
    # small tensors go via the ACT HWDGE ring so their per-DMA completion
    # latency doesn't serialize in front of x.
    x_sb = pp.tile([128, 4, T], F32, tag="x")
    x_r = d_x.rearrange("(j p) t -> p j t", p=128)
    for j in range(4):
        nc.sync.dma_start(x_sb[:, j, :], x_r[:, j, :])
    gmat_sb = wl.tile([128, 8], F32, tag="gmat")
    nc.scalar.dma_start(gmat_sb[:], d_w[:, OFF_GMAT:OFF_GMAT + 16].bitcast(F32))
    gmt_sb = wl.tile([128, 128], F32, tag="gmt")
    nc.scalar.dma_start(gmt_sb[:], d_w[:, OFF_GMT:OFF_GMT + 256].bitcast(F32))
    gnw_sb = wl.tile([128, 4], F32, tag="gnw")
    nc.scalar.dma_start(gnw_sb[:], d_w[:, OFF_GNW:OFF_GNW + 8].bitcast(F32))
    gnb_sb = wl.tile([128, 4], F32, tag="gnb")
    nc.scalar.dma_start(gnb_sb[:], d_w[:, OFF_GNB:OFF_GNB + 8].bitcast(F32))
    bqk_sb = wl.tile([128, 8], F32, tag="bqk")
    nc.scalar.dma_start(bqk_sb[:], d_w[:, OFF_BQK:OFF_BQK + 16].bitcast(F32))
    bp_sb = wl.tile([128, 4], F32, tag="bp")
    nc.scalar.dma_start(bp_sb[:], d_w[:, OFF_BP:OFF_BP + 8].bitcast(F32))
    ones_sb = wl.tile([1, 128], BF16, tag="ones")
    nc.vector.memset(ones_sb[:], 1.0)
    bv_sb = wl.tile([1, C], BF16, tag="bv")
    nc.scalar.dma_start(bv_sb[:], d_w[0:1, OFF_BV:OFF_BV + 512])
    # weights split per K-chunk so the first matmuls start early
    wqk_sb = wl.tile([128, 4, 2 * C], BF16, tag="wqk")
    wv_sb = wl.tile([128, 4, C], BF16, tag="wv")
    wp_sb = wl.tile([128, 4, C], BF16, tag="wp")
    for kc in range(4):
        nc.sync.dma_start(
            wqk_sb[:, kc, :], d_w[:, OFF_WQK + 1024 * kc:OFF_WQK + 1024 * (kc + 1)])
        nc.sync.dma_start(
            wv_sb[:, kc, :], d_w[:, OFF_WV + 512 * kc:OFF_WV + 512 * (kc + 1)])
    for kc in range(4):
        nc.sync.dma_start(
            wp_sb[:, kc, :], d_w[:, OFF_WP + 512 * kc:OFF_WP + 512 * (kc + 1)])

    xn_sb = pp.tile([128, 4, T], BF16, tag="xn")
    qk_sb = pp.tile([128, 8, T], BF16, tag="qk")
    vt_sb = pp.tile([128, 8, V], BF16, tag="vt")
    a_sb = pp.tile([128, 4, T], BF16, tag="a")

    # ================= GroupNorm =================
    # Groups (16 channels) never span a 128-channel chunk, so each chunk's
    # GN runs independently -> xn chunk j ready as soon as x chunk j lands.
    stats = pp.tile([128, 4, 2, 6], F32, tag="stats")
    mv = pp.tile([128, 4, 2], F32, tag="mv")
    packed = pp.tile([128, 4, 2], F32, tag="packed")
    msq = pp.tile([128, 4], F32, tag="msq")
    gstats = pp.tile([8, 4, 2], F32, tag="gstats")
    msqg = pp.tile([8, 4], F32, tag="msqg")
    varg = pp.tile([8, 4], F32, tag="varg")
    lnv = pp.tile([8, 4], F32, tag="lnv")
    bcin = pp.tile([128, 4, 2], F32, tag="bcin")
    nc.vector.memset(bcin[:], 0.0)
    eps_sb = wl.tile([8, 1], F32, tag="eps")
    nc.vector.memset(eps_sb[:], EPS)
    # dummy Ln: pulls the ACT table load off the GN critical path
    dum_sb = wl.tile([1, 1], F32, tag="dum")
    nc.scalar.activation(dum_sb[:], eps_sb[0:1, 0:1], ACTF.Ln)
    A_sb = pp.tile([128, 4], F32, tag="A")
    t1_sb = pp.tile([128, 4], F32, tag="t1")
    B_sb = pp.tile([128, 4], F32, tag="B")
    for j in range(4):
        for sg in range(2):
            nc.vector.bn_stats(stats[:, j, sg, :], x_sb[:, j, 512 * sg:512 * (sg + 1)])
        nc.vector.bn_aggr(mv[:, j, :], stats[:, j, :, :])
        # packed per-channel (mean, E[x^2])
        nc.vector.tensor_copy(packed[:, j, 0:1], mv[:, j, 0:1])
        nc.vector.tensor_mul(msq[:, j:j + 1], mv[:, j, 0:1], mv[:, j, 0:1])
        nc.vector.tensor_add(packed[:, j, 1:2], msq[:, j:j + 1], mv[:, j, 1:2])
        # per-group (mean, E[x^2]) via 0/1-matrix matmul (K=128 over chans)
        gp = ps.tile([8, 2], F32, tag="sc", name=f"gn{j}")
        nc.tensor.matmul(gp[:], gmat_sb[:], packed[:, j, :], start=True, stop=True)
        nc.vector.tensor_copy(gstats[:, j, :], gp[:])
        # rstd = exp(-0.5*ln(var+eps)); var = E[x^2]-mean^2
        nc.vector.tensor_mul(msqg[:, j:j + 1], gstats[:, j, 0:1], gstats[:, j, 0:1])
        nc.vector.tensor_sub(varg[:, j:j + 1], gstats[:, j, 1:2], msqg[:, j:j + 1])
        # Ln for all chunks back-to-back (one ACT table), Exp batched below
        nc.scalar.activation(lnv[:, j:j + 1], varg[:, j:j + 1], ACTF.Ln, bias=eps_sb[:])
        nc.vector.tensor_copy(bcin[0:8, j, 0:1], gstats[:, j, 0:1])
    for j in range(4):
        nc.scalar.activation(bcin[0:8, j, 1:2], lnv[:, j:j + 1], ACTF.Exp, scale=-0.5)
        # broadcast group stats back to channels; xn = x*A + B
        bb = ps.tile([128, 2], F32, tag="sc", name=f"gb{j}")
        nc.tensor.matmul(bb[:], gmt_sb[:], bcin[:, j, :], start=True, stop=True)
        nc.vector.tensor_mul(A_sb[:, j:j + 1], bb[:, 1:2], gnw_sb[:, j:j + 1])
        nc.vector.tensor_mul(t1_sb[:, j:j + 1], bb[:, 0:1], A_sb[:, j:j + 1])
        nc.vector.tensor_sub(B_sb[:, j:j + 1], gnb_sb[:, j:j + 1], t1_sb[:, j:j + 1])
        nc.vector.tensor_scalar(
            xn_sb[:, j, :], x_sb[:, j, :],
            A_sb[:, j:j + 1], B_sb[:, j:j + 1], op0=ALU.mult, op1=ALU.add)

    # ================= QKV / attention (interleaved) =================
    def qk_group(jo, th):
        pq = ps.tile([128, 512], F32, tag="sc", name=f"qk{jo}_{th}")
        for kc in range(4):
            nc.tensor.matmul(
                pq[:], wqk_sb[:, kc, 128 * jo:128 * (jo + 1)],
                xn_sb[:, kc, 512 * th:512 * (th + 1)],
                start=(kc == 0), stop=(kc == 3))
        nc.vector.tensor_scalar_add(
            qk_sb[:, jo, 512 * th:512 * (th + 1)], pq[:], bqk_sb[:, jo:jo + 1])

    def qk_chunk(jo):
        # o-chunk jo: 0-3 = q heads (2jo,2jo+1) pre-scaled, 4-7 = k heads
        for th in range(2):
            qk_group(jo, th)

    # per-head ones columns (PV lhsT col 64 -> softmax denominator) are a
    # constant: one strided memset instead of 40 extra matmuls
    vt4 = vt_sb[:].rearrange("p j (h c) -> p j h c", c=65)
    nc.vector.memset(vt4[:, :, :, 64:65], 1.0)

    def v_group(jt):
        # v^T + bias via K=1 matmul; strided copy into the 65-col layout
        pv_ = ps.tile([128, 512], F32, tag="sc", name=f"v{jt}")
        for kc in range(4):
            nc.tensor.matmul(
                pv_[:], xn_sb[:, kc, 128 * jt:128 * (jt + 1)],
                wv_sb[:, kc, :], start=(kc == 0), stop=False)
        nc.tensor.matmul(pv_[:], ones_sb[:], bv_sb[:], start=False, stop=True)
        nc.vector.tensor_copy(
            vt4[:, jt, :, 0:64],
            pv_[:].rearrange("p (h c) -> p h c", c=64))

    def head_tail(h, pv, split=False):
        hp = h // 2
        den = tl.tile([1, T], F32, tag="den", name=f"dn{h}")
        rden = tl.tile([1, T], F32, tag="rden", name=f"rd{h}")
        bc = tl.tile([64, T], F32, tag="bc", name=f"bc{h}")
        stg = None
        if h % 2 == 1:
            stg = tl.tile([64, T], BF16, tag="astage", name=f"ast{h}")
        halves = ((0, 512), (512, T)) if split else ((0, T),)
        for lo, hi in halves:
            nc.vector.tensor_copy(den[:, lo:hi], pv[64:65, lo:hi])
            nc.vector.reciprocal_approx_fast(out=rden[:, lo:hi], in_=den[0:1, lo:hi])
            nc.gpsimd.partition_broadcast(bc[:, lo:hi], rden[0:1, lo:hi], channels=64)
            if h % 2 == 0:
                nc.vector.tensor_mul(a_sb[0:64, hp, lo:hi], pv[0:64, lo:hi], bc[:, lo:hi])
            else:
                nc.vector.tensor_mul(stg[:, lo:hi], pv[0:64, lo:hi], bc[:, lo:hi])
                nc.sync.dma_start(a_sb[64:128, hp, lo:hi], stg[:, lo:hi])

    def attn_pair(hp, fill=None, late=None):
        # Two heads ping-pong chunk-wise: while ACT runs head A's exp, the
        # PE does head B's scores (and vice versa), so the cross-engine
        # handoff latency is hidden. The two heads' k/q live at partitions
        # 0-63 / 64-127, so their K=64 score matmuls land in different PE
        # row groups and run concurrently. PV is software-pipelined one
        # s-chunk behind. `fill` thunks emit independent PE work (qkv/proj
        # groups) to fill PE wait-slots.
        heads = (2 * hp, 2 * hp + 1)
        pvt = [pa.tile([65, T], F32, tag="pv", name=f"pv{h}") for h in heads]
        prev = [None, None]
        fill = list(fill or [])
        late = list(late or [])
        for sj in range(8):
            cur = []
            for i, h in enumerate(heads):
                off = 64 * (h % 2)
                sc = ps.tile([128, T], F32, tag="sc", name=f"sc{h}_{sj}")
                for th in range(2):
                    nc.tensor.matmul(
                        sc[:, 512 * th:512 * (th + 1)],
                        qk_sb[off:off + 64, 4 + hp, 128 * sj:128 * (sj + 1)],
                        qk_sb[off:off + 64, hp, 512 * th:512 * (th + 1)],
                        start=True, stop=True)
                es = wk.tile([128, T], BF16, tag="es", name=f"es{h}_{sj}")
                nc.scalar.activation(es[:], sc[:], ACTF.Exp)
                cur.append(es)
            for i, h in enumerate(heads):
                if prev[i] is not None:
                    for th in range(2):
                        nc.tensor.matmul(
                            pvt[i][:, 512 * th:512 * (th + 1)],
                            vt_sb[:, sj - 1, 65 * h:65 * h + 65],
                            prev[i][:, 512 * th:512 * (th + 1)],
                            start=(sj - 1 == 0), stop=False)
                prev[i] = cur[i]
            n = -(-len(fill) // (8 - sj)) if fill else 0
            for _ in range(n):
                fill.pop(0)()
            # late fills depend on the previous pair's tail chain; hold
            # them until sj>=2 so the in-order PE never stalls on them
            if sj >= 2 and late:
                n = -(-len(late) // (8 - sj))
                for _ in range(n):
                    late.pop(0)()
        for i, h in enumerate(heads):
            for th in range(2):
                nc.tensor.matmul(
                    pvt[i][:, 512 * th:512 * (th + 1)],
                    vt_sb[:, 7, 65 * h:65 * h + 65],
                    prev[i][:, 512 * th:512 * (th + 1)],
                    start=False, stop=True)
        for i, h in enumerate(heads):
            head_tail(h, pvt[i], split=(hp == 3))

    # ---- progressive proj: after head pair hp completes a[:, hp, :],
    # fold its contribution into out_sb so only pair 3's work remains at
    # the end. Pair 0 also folds in the residual x and proj bias.
    out_sb = pp.tile([128, 4, T], F32, tag="out")
    out_r = d_out.rearrange("(j p) t -> p j t", p=128)

    def proj_grp(hp, jo, th):
        pj = ps.tile([128, 512], F32, tag="sc", name=f"pj{hp}_{jo}_{th}")
        nc.tensor.matmul(
            pj[:], wp_sb[:, hp, 128 * jo:128 * (jo + 1)],
            a_sb[:, hp, 512 * th:512 * (th + 1)],
            start=True, stop=True)
        dst = out_sb[:, jo, 512 * th:512 * (th + 1)]
        if hp == 0:
            nc.vector.scalar_tensor_tensor(
                dst, pj[:], bp_sb[:, jo:jo + 1],
                x_sb[:, jo, 512 * th:512 * (th + 1)],
                op0=ALU.add, op1=ALU.add)
        else:
            nc.vector.tensor_add(dst, dst, pj[:])
        if hp == 3 and th == 1:
            nc.sync.dma_start(out_r[:, jo, :], out_sb[:, jo, :])

    for hp in range(4):
        fill = []
        late = []
        if hp == 0:
            qk_chunk(0)
            qk_chunk(4)
            # v^T groups ride in the pair's fill slots: vt[:, jt] lands
            # one s-chunk ahead of the PV matmuls that consume it
            fill += [lambda jt=jt: v_group(jt) for jt in range(8)]
        else:
            # fold the previous pair's proj contribution while this pair's
            # attention runs; a[:, hp-1] needs the previous tails, so these
            # go in the LATE class
            late += [lambda jo=jo, th=th, p=hp - 1: proj_grp(p, jo, th)
                     for jo in range(4) for th in range(2)]
        if hp < 3:
            fill += [lambda th=th, jo=jo: qk_group(jo, th)
                     for jo in (hp + 1, 4 + hp + 1) for th in range(2)]
        attn_pair(hp, fill=fill, late=late)
    for jo in range(4):
        for th in range(2):
            proj_grp(3, jo, th)


# ----------------------------------------------------------------------------
# Host side
# ----------------------------------------------------------------------------

def _chunked_T(a):
    """[512, n] -> [128, 4*n] with chunk-major layout a[j*128+p, o] -> [p, j*n+o]."""
    n = a.shape[1]
    return np.ascontiguousarray(
        a.reshape(4, 128, n).transpose(1, 0, 2).reshape(128, 4 * n))


def _colmajor128(a):
    """[k*128] -> [128, k] with a[j*128+p] -> [p, j]."""
    k = a.shape[0] // 128
    return np.ascontiguousarray(a.reshape(k, 128).T)


def make_in_maps(x, norm_w, norm_b, qkv_w, qkv_b, proj_w, proj_b):
    B = x.shape[0]
    bf = ml_dtypes.bfloat16
    xf = np.ascontiguousarray(x.reshape(B, C, T), dtype=np.float32)
    qkv_w = np.asarray(qkv_w, dtype=np.float32)
    qkv_b = np.asarray(qkv_b, dtype=np.float32)

    s2 = np.float32(1.0 / np.sqrt(CH))  # scale^2 = 1/8, folded into q
    q_rows = np.concatenate([np.arange(192 * h, 192 * h + 64) for h in range(NH)])
    k_rows = np.concatenate([np.arange(192 * h + 64, 192 * h + 128) for h in range(NH)])
    v_rows = np.concatenate([np.arange(192 * h + 128, 192 * h + 192) for h in range(NH)])

    wqk = np.empty((C, 2 * C), dtype=np.float32)
    wqk[:, :C] = qkv_w[q_rows].T * s2
    wqk[:, C:] = qkv_w[k_rows].T
    bqk = np.concatenate([qkv_b[q_rows] * s2, qkv_b[k_rows]]).astype(np.float32)

    wv = np.ascontiguousarray(qkv_w[v_rows].T)
    bv = qkv_b[v_rows].astype(np.float32)

    wp = np.ascontiguousarray(np.asarray(proj_w, dtype=np.float32).T)
    bp = np.asarray(proj_b, dtype=np.float32)

    gmat = np.zeros((128, 8), dtype=np.float32)
    for p in range(128):
        gmat[p, p // 16] = 1.0 / 16.0
    gmt = np.zeros((128, 128), dtype=np.float32)
    for p in range(128):
        gmt[p // 16, p] = 1.0

    def f32cols(a):
        # f32 [128, n] -> raw bf16 [128, 2n] (bitcast, little-endian pairs)
        return np.ascontiguousarray(a, dtype=np.float32).view(bf)

    wpack = np.zeros((128, WCOLS), dtype=bf)
    wpack[:, OFF_WQK:OFF_WQK + 4096] = _chunked_T(wqk).astype(bf)
    wpack[:, OFF_WV:OFF_WV + 2048] = _chunked_T(wv).astype(bf)
    wpack[:, OFF_WP:OFF_WP + 2048] = _chunked_T(wp).astype(bf)
    wpack[0, OFF_BV:OFF_BV + 512] = bv.astype(bf)
    wpack[:, OFF_GMT:OFF_GMT + 256] = f32cols(gmt)
    wpack[:, OFF_GMAT:OFF_GMAT + 16] = f32cols(gmat)
    wpack[:, OFF_GNW:OFF_GNW + 8] = f32cols(_colmajor128(np.asarray(norm_w, np.float32)))
    wpack[:, OFF_GNB:OFF_GNB + 8] = f32cols(_colmajor128(np.asarray(norm_b, np.float32)))
    wpack[:, OFF_BQK:OFF_BQK + 16] = f32cols(_colmajor128(bqk))
    wpack[:, OFF_BP:OFF_BP + 8] = f32cols(_colmajor128(bp))

    return [dict(w=wpack, x=xf[b]) for b in range(B)]


_NC_CACHE = []


def _get_nc():
    if not _NC_CACHE:
        _NC_CACHE.append(build_bass())
    return _NC_CACHE[0]


def kernel(x, norm_w, norm_b, qkv_w, qkv_b, proj_w, proj_b):
    x = np.asarray(x)
    B, _, H, W = x.shape
    in_maps = make_in_maps(x, norm_w, norm_b, qkv_w, qkv_b, proj_w, proj_b)
    nc = _get_nc()
    res = run_bass_kernel_spmd(nc, in_maps, core_ids=list(range(N_CORES)))
    out = np.stack([res.results[b]["out"].reshape(C, H, W) for b in range(B)])
    return out.astype(np.float32)
